# revision 112
# baseline (speedup 1.0000x reference)
"""Multi-head attention (B=2, S=2048, D=1024, H=16, dk=64) on 8 TRN2 cores.

Sharding: core c -> (batch b = c//4, head-group g = c%4 of 4 heads).

Design (ACT-paced pipeline): the softmax exp is the hard floor -- 4 heads
x 2048^2 scores / 128 lanes on the only engine that can exp (ACT) is
~133us.  Everything else is scheduled to hide under it:
  - inputs stream in 256-column s-slices (full-bandwidth 4KB runs) so the
    first exp fires at ~12us instead of ~26us;
  - per step (pair-major, 64 steps): scoresT[j,i] (2 x [128,1024] PSUM)
    -> exp -> probs[j,i] bf16 in SBUF (ring of 40, they are re-read for
    the denominators);
  - PV in [i,e] orientation: acc[i,64] += probs[j,i-chunk]^T @ v[j,64] --
    F=64 accumulating matmuls, half the PE rows of the [e,i] form.  PSUM
    allows only ONE in-flight accumulation chain per bank, so each
    (head,ic) chain runs region-major to completion; chains are split at
    jt=10 (part1 stashed to SBUF) so the tail pair's work shrinks;
  - denominators ride as F=1 ones-matmul chains (nearly free);
  - normalize = reciprocal + per-partition scalar-mul; one [128,128] PE
    transpose per 128-column block covers both heads -> attT[e,s];
  - out projection K=128x2 from attT; first s-half overlaps attention,
    final 8 tiles pipeline per-ic with the last boundary; bf16 partial
    outputs (host reduces in f32);
  - a ns-budget list scheduler emits filler units (projections, drains,
    out-proj) against per-step deadlines derived from DMA-ring recycling
    order -- consumers of slice s must all be emitted before any consumer
    of slice s+bufs, or the serial DMA queue deadlocks.
  PSUM: scores 2x[128,1024](4 banks) + pvacc 2x[128,512](2) + work ring
  2x[128,512](2) = 8 banks exactly.
"""

import os
from collections import deque

import numpy as np
import ml_dtypes

BF16 = ml_dtypes.bfloat16

B, S, D = 2, 2048, 1024
H, DK = 16, 64
P = 128
GROUPS = 4
HPG = 4             # heads per group (2 pairs)
GD = HPG * DK       # 256
KC = D // P         # 8 contraction chunks
NSL = 8             # 256-wide s-slices per tensor
SLW = S // NSL      # 256
ST = S // P         # 16 j-tiles / s-tiles
NCORES = 8

_cached = {}
DEBUG = False
# schedule knobs (emission order only — numerics-neutral)
KNOBS = {
    'step_budget': 1800.0,  # ns of non-score PE work per step
    'forced_age': 1,        # steps overdue before a filler bypasses budget
    'drain_age': 2,         # drain age that bypasses budget (probs ring)
    'drain_cap': 6,
    'jsplit': 10,           # PV chain split point (part1 = jt<jsplit)
    'q47_t0': 10,            # deadline base for qproj t0 slices 4-7 (+sl)
    'q47_t1': 38,           # deadline base for qproj t1 slices 4-7 (+sl)
    'late_start': 52,       # step to start outproj st0-7
    'vp_base': 11,           # vproj jt<12 deadline base (+jt//2)
}


def _build_bass():
    import concourse.bass as bass
    import concourse.tile as tile
    from concourse.bacc import Bacc
    from concourse import mybir
    from contextlib import ExitStack

    f32 = mybir.dt.float32
    bf16 = mybir.dt.bfloat16
    Act = mybir.ActivationFunctionType

    nc = Bacc()

    xq = nc.dram_tensor("xq", [P, NSL * KC * SLW], bf16, kind="ExternalInput")
    xk = nc.dram_tensor("xk", [P, NSL * KC * SLW], bf16, kind="ExternalInput")
    xv = nc.dram_tensor("xv", [P, NSL * KC * SLW], bf16, kind="ExternalInput")
    wq = nc.dram_tensor("wq", [P, KC * GD], bf16, kind="ExternalInput")
    wk = nc.dram_tensor("wk", [P, KC * GD], bf16, kind="ExternalInput")
    wv = nc.dram_tensor("wv", [P, KC * GD], bf16, kind="ExternalInput")
    wo = nc.dram_tensor("wo", [P, 2 * D], bf16, kind="ExternalInput")
    bq = nc.dram_tensor("bq", [P, 2], f32, kind="ExternalInput")
    bk = nc.dram_tensor("bk", [P, 2], f32, kind="ExternalInput")
    ident = nc.dram_tensor("ident", [P, P], bf16, kind="ExternalInput")
    out = nc.dram_tensor("out", [S, D], bf16, kind="ExternalOutput")
    if DEBUG:
        dbg = {
            'qT0': nc.dram_tensor("d_qT0", [P, S], bf16, kind="ExternalOutput"),
            'kT0': nc.dram_tensor("d_kT0", [P, S], bf16, kind="ExternalOutput"),
            'v': nc.dram_tensor("d_v", [P, ST * HPG * DK], bf16,
                                kind="ExternalOutput"),
            'pr000': nc.dram_tensor("d_pr000", [P, 1024], bf16,
                                    kind="ExternalOutput"),
            'rec00': nc.dram_tensor("d_rec00", [P, 16], f32,
                                    kind="ExternalOutput"),
            'pv00': nc.dram_tensor("d_pv00", [P, 2 * 8 * DK], f32,
                                   kind="ExternalOutput"),
            'attT0': nc.dram_tensor("d_attT0", [P, S], bf16,
                                    kind="ExternalOutput"),
            'attT1': nc.dram_tensor("d_attT1", [P, S], bf16,
                                    kind="ExternalOutput"),
        }

    with tile.TileContext(nc) as tc, ExitStack() as ctx:
        singles = ctx.enter_context(tc.tile_pool(name="singles", bufs=1))
        xring = ctx.enter_context(tc.tile_pool(name="xring", bufs=4))
        probs_pool = ctx.enter_context(tc.tile_pool(name="probs", bufs=40))
        attn_pool = ctx.enter_context(tc.tile_pool(name="attn", bufs=8))
        rec_pool = ctx.enter_context(tc.tile_pool(name="rec", bufs=2))
        outs_pool = ctx.enter_context(tc.tile_pool(name="outs", bufs=4))
        psum = ctx.enter_context(tc.tile_pool(name="psum", bufs=1, space="PSUM"))

        # ---------------- persistent SBUF ----------------
        wq_sb = singles.tile([P, KC, GD], bf16)
        wk_sb = singles.tile([P, KC, GD], bf16)
        wv_sb = singles.tile([P, KC, GD], bf16)
        wo_sb = singles.tile([P, 2, D], bf16)
        bq_sb = singles.tile([P, 2], f32)
        bk_sb = singles.tile([P, 2], f32)
        ident_sb = singles.tile([P, P], bf16)
        ones_sb = singles.tile([P, 1], bf16)
        dummy_sb = singles.tile([P, 1], f32)

        qT_sb = [singles.tile([P, S], bf16, name=f"qT{t}") for t in range(2)]
        kT_sb = [singles.tile([P, S], bf16, name=f"kT{t}") for t in range(2)]
        v_sb = singles.tile([P, ST, HPG, DK], bf16)
        attT = [singles.tile([P, S], bf16, name=f"attT{p}") for p in range(2)]

        # warm the ACT exp table during the DMA lead-in
        nc.vector.memset(ones_sb, 1.0)
        nc.vector.memset(dummy_sb, 0.0)
        warm = singles.tile([P, 1], f32)
        nc.scalar.activation(out=warm, in_=dummy_sb, func=Act.Exp)

        # ---------------- DMA stream (order = schedule) ----------------
        nc.sync.dma_start(out=wq_sb, in_=wq.rearrange("p (c m) -> p c m", c=KC))

        # x slice rings: slot = [P, KC, SLW] bf16 (4KB/partition)
        x_slots = {}

        def dma_x(which, dram, sl):
            t = xring.tile([P, KC, SLW], bf16, tag=f"x{which}",
                           bufs={'q': 4, 'k': 8, 'v': 4}[which],
                           name=f"x{which}{sl}")
            nc.sync.dma_start(
                out=t, in_=dram.rearrange("p (sl c s) -> p sl c s",
                                          sl=NSL, c=KC)[:, sl])
            x_slots[(which, sl)] = t

        dma_x('q', xq, 0)
        nc.sync.dma_start(out=wk_sb, in_=wk.rearrange("p (c m) -> p c m", c=KC))
        dma_x('q', xq, 1)
        dma_x('k', xk, 0)
        nc.sync.dma_start(out=bq_sb, in_=bq[:, :])
        nc.sync.dma_start(out=bk_sb, in_=bk[:, :])
        dma_x('q', xq, 2)
        dma_x('q', xq, 3)
        dma_x('k', xk, 1)
        dma_x('k', xk, 2)
        nc.sync.dma_start(out=wv_sb, in_=wv.rearrange("p (c m) -> p c m", c=KC))
        for g in range(3, 8):
            dma_x('k', xk, g)
        nc.sync.dma_start(out=ident_sb, in_=ident[:, :])
        for g in range(8):
            dma_x('v', xv, g)
        for sl in range(4, 8):
            dma_x('q', xq, sl)
        nc.sync.dma_start(out=wo_sb, in_=wo.rearrange("p (c n) -> p c n", c=2))

        # ---------------- unit emitters (memoized) ----------------
        emitted = set()
        vclock = [0.0]     # virtual PE ns

        def mm_cost(rows, n=1):
            return rows * 0.42 + n * 5.0

        def qkproj(which, t, sl):
            key = (which, t, sl)
            if key in emitted:
                return
            emitted.add(key)
            x_t = x_slots[(which, sl)]
            w_sb = wq_sb if which == 'q' else wk_sb
            b_sb = bq_sb if which == 'q' else bk_sb
            dstT = (qT_sb if which == 'q' else kT_sb)[t]
            pp = psum.tile([P, 512], f32, tag="work", bufs=2, name="pp")
            for c in range(KC):
                nc.tensor.matmul(out=pp[:, 0:SLW],
                                 lhsT=w_sb[:, c, t * P:(t + 1) * P],
                                 rhs=x_t[:, c, :],
                                 start=(c == 0), stop=(c == KC - 1))
            nc.vector.tensor_scalar_add(
                out=dstT[:, sl * SLW:(sl + 1) * SLW],
                in0=pp[:, 0:SLW], scalar1=b_sb[:, t:t + 1])
            vclock[0] += mm_cost(KC * SLW, KC)

        def vproj(jt, pair=None):
            key = ('v', jt)
            if key in emitted:
                return
            emitted.add(key)
            sl, half = jt // 2, jt % 2
            x_t = x_slots[('v', sl)]
            pp = psum.tile([P, 512], f32, tag="work", bufs=2, name="pv")
            for c in range(KC):
                nc.tensor.matmul(
                    out=pp[:, 0:GD],
                    lhsT=x_t[:, c, half * P:(half + 1) * P],
                    rhs=wv_sb[:, c, :],
                    start=(c == 0), stop=(c == KC - 1))
            nc.vector.tensor_copy(
                out=v_sb[:, jt], in_=pp[:, 0:GD].rearrange("p (h e) -> p h e",
                                                           e=DK))
            vclock[0] += mm_cost(KC * GD, KC)

        probs_tiles = {}
        pvaccs = {}
        sc_seq = [0]

        def scores_exp(ih, pair, jt):
            qkproj('k', pair, jt // 2)
            for sl in range(ih * 4, ih * 4 + 4):
                qkproj('q', pair, sl)
            for hp in range(2):
                sc = psum.tile([P, 1024], f32, tag="sc", bufs=2,
                               name=f"sc{sc_seq[0] % 2}")
                sc_seq[0] += 1
                for iq in range(2):
                    nc.tensor.matmul(
                        out=sc[:, iq * 512:(iq + 1) * 512],
                        lhsT=kT_sb[pair][hp * DK:(hp + 1) * DK,
                                         jt * P:(jt + 1) * P],
                        rhs=qT_sb[pair][hp * DK:(hp + 1) * DK,
                                        ih * 1024 + iq * 512:
                                        ih * 1024 + (iq + 1) * 512],
                        start=True, stop=True)
                pr = probs_pool.tile([P, 1024], bf16, tag="probs", name="pr")
                nc.scalar.activation(out=pr, in_=sc, func=Act.Exp, scale=0.125)
                probs_tiles[(ih, pair, jt, hp)] = pr
                vclock[0] += mm_cost(1024, 2)
                if DEBUG and (ih, pair, jt, hp) == (0, 0, 0, 0):
                    nc.sync.dma_start(out=dbg['pr000'][:, :], in_=pr)

        # PV + denominators run as per-(hp,ic) accumulation chains, split at
        # jt=JSPLIT: part1 runs inside the pair's own late steps (stashed to
        # SBUF), part2 + combine after the pair's scores finish.  PSUM rule:
        # only one in-flight chain per bank, sequential chains are fine.
        JSPLIT = int(KNOBS.get('jsplit', 12))
        dstate = {}
        in_tail = [False]   # when True (ACT idle), split copies ACT/DVE
        stage_pool = ctx.enter_context(tc.tile_pool(name="stage", bufs=1))

        def _pair_state(ih, pair):
            key = (ih, pair)
            if key not in dstate:
                acc = [psum.tile([P, 8, DK], f32, tag="pvacc", bufs=2,
                                 name=f"pva{hp2}") for hp2 in range(2)]
                rec = rec_pool.tile([P, 16], f32, tag="rec", name="rec")
                stage = stage_pool.tile([P, 16, DK], f32, tag="stg",
                                        name="stage")
                d1 = stage_pool.tile([P, 16], f32, tag="d1", name="d1")
                dstate[key] = (acc, rec, stage, d1)
            return dstate[key]

        def part1_unit(ih, pair, hp, ic):
            acc, rec, stage, d1 = _pair_state(ih, pair)
            h = 2 * pair + hp
            idx = hp * 8 + ic
            den = psum.tile([P, 1], f32, tag="work", bufs=2, name="den")
            for jt in range(JSPLIT):
                vproj(jt, pair)
                pr = probs_tiles[(ih, pair, jt, hp)]
                nc.tensor.matmul(
                    out=acc[hp][:, ic],
                    lhsT=pr[:, ic * P:(ic + 1) * P],
                    rhs=v_sb[:, jt, h],
                    start=(jt == 0), stop=(jt == JSPLIT - 1))
                nc.tensor.matmul(
                    out=den,
                    lhsT=pr[:, ic * P:(ic + 1) * P],
                    rhs=ones_sb,
                    start=(jt == 0), stop=(jt == JSPLIT - 1))
            nc.vector.tensor_copy(out=stage[:, idx], in_=acc[hp][:, ic])
            nc.vector.tensor_copy(out=d1[:, idx:idx + 1], in_=den)
            vclock[0] += mm_cost(JSPLIT * (DK + 1), 2 * JSPLIT)

        def drain_unit(ih, pair, hp, ic):
            acc, rec, stage, d1 = _pair_state(ih, pair)
            h = 2 * pair + hp
            idx = hp * 8 + ic
            den = psum.tile([P, 1], f32, tag="work", bufs=2, name="den")
            for jt in range(JSPLIT, ST):
                vproj(jt, pair)
                pr = probs_tiles[(ih, pair, jt, hp)]
                nc.tensor.matmul(
                    out=acc[hp][:, ic],
                    lhsT=pr[:, ic * P:(ic + 1) * P],
                    rhs=v_sb[:, jt, h],
                    start=(jt == JSPLIT), stop=(jt == ST - 1))
                nc.tensor.matmul(
                    out=den,
                    lhsT=pr[:, ic * P:(ic + 1) * P],
                    rhs=ones_sb,
                    start=(jt == JSPLIT), stop=(jt == ST - 1))
            dsum = stage_pool.tile([P, 1], f32, tag="dsum", bufs=2,
                                   name="dsum")
            nc.vector.tensor_add(out=dsum, in0=den, in1=d1[:, idx:idx + 1])
            nc.vector.reciprocal(out=rec[:, idx:idx + 1], in_=dsum)
            vclock[0] += mm_cost((ST - JSPLIT) * (DK + 1), 2 * (ST - JSPLIT))

        def finish_ic(ih, pair, ic):
            # combine part1+part2, normalize + transpose one 128-column block
            acc, rec, stage, d1 = dstate[(ih, pair)]
            at = attn_pool.tile([P, P], bf16, tag="attn", name="at")
            for hp in range(2):
                idx = hp * 8 + ic
                r = rec[:, idx:idx + 1]
                tsum = stage_pool.tile([P, DK], f32, tag="tsum", bufs=2,
                                       name="tsum")
                nc.vector.tensor_add(out=tsum, in0=acc[hp][:, ic],
                                     in1=stage[:, idx])
                if in_tail[0] and hp == 1:
                    nc.scalar.mul(out=at[:, hp * DK:(hp + 1) * DK],
                                  in_=tsum, mul=r)
                else:
                    nc.vector.tensor_scalar_mul(
                        out=at[:, hp * DK:(hp + 1) * DK],
                        in0=tsum, scalar1=r)
            tp = psum.tile([P, P], bf16, tag="work", bufs=2, name="tp")
            nc.tensor.transpose(out=tp, in_=at, identity=ident_sb)
            col = (ih * 8 + ic) * P
            if in_tail[0] and ic % 2 == 1:
                nc.scalar.copy(out=attT[pair][:, col:col + P], in_=tp)
            else:
                nc.vector.tensor_copy(out=attT[pair][:, col:col + P], in_=tp)
            if in_tail[0]:
                outproj(ih * 8 + ic)
            vclock[0] += mm_cost(P, 1)

        def boundary(ih, pair):
            acc, rec, stage, d1 = dstate.pop((ih, pair))
            if DEBUG and (ih, pair) == (0, 0):
                nc.sync.dma_start(out=dbg['rec00'][:, :], in_=rec)
            for jt in range(ST):
                for hp in range(2):
                    del probs_tiles[(ih, pair, jt, hp)]

        def outproj(st):
            osb = outs_pool.tile([P, D], bf16, tag="osb", name="osb")
            for nb in range(2):
                # in the tail the sc ring is idle; using it keeps the work
                # ring free for the drain/transpose tiles
                po = psum.tile([P, 512], f32,
                               tag="sc" if in_tail[0] else "work",
                               bufs=2, name="po")
                for c in range(2):
                    nc.tensor.matmul(
                        out=po,
                        lhsT=attT[c][:, st * P:(st + 1) * P],
                        rhs=wo_sb[:, c, nb * 512:(nb + 1) * 512],
                        start=(c == 0), stop=(c == 1))
                col = nb * 512
                if in_tail[0]:
                    # halve the copy latency: ACT and DVE take one half each
                    nc.scalar.copy(out=osb[:, col:col + 256], in_=po[:, 0:256])
                    nc.vector.tensor_copy(out=osb[:, col + 256:col + 512],
                                          in_=po[:, 256:512])
                else:
                    nc.vector.tensor_copy(out=osb[:, col:col + 512], in_=po)
                if in_tail[0]:
                    nc.sync.dma_start(
                        out=out[st * P:(st + 1) * P, col:col + 512],
                        in_=osb[:, col:col + 512])
            if not in_tail[0]:
                nc.sync.dma_start(out=out[st * P:(st + 1) * P, :], in_=osb)
            vclock[0] += mm_cost(2 * 512, 4)

        # ---------------- schedule ----------------

        # fillers: (deadline_step, est_cost_ns, emit_fn), deadline-sorted
        U = 900.0   # est ns for one 8-chunk projection unit
        fillers = deque()
        for g in range(1, 8):
            fillers.append((2 * g - 1, U, lambda g=g: qkproj('k', 0, g)))
        # xq ring order: t1's early slices must follow t0's closely; steps
        # 1-2 have slack (scores only)
        fillers.append((1, U, lambda: qkproj('q', 1, 2)))
        fillers.append((2, U, lambda: qkproj('q', 1, 3)))
        fillers.append((30, U, lambda: qkproj('k', 1, 0)))
        for g in range(1, 8):
            fillers.append((30 + 2 * g, U, lambda g=g: qkproj('k', 1, g)))
        # xv ring (4 slots): slice v+4 arrives ~step 8 and reuses slot 0
        for jt in range(JSPLIT):
            fillers.append((KNOBS['vp_base'] + jt // 2, U,
                            lambda jt=jt: vproj(jt)))
        for jt in range(JSPLIT, ST):
            fillers.append((13 + (jt - JSPLIT), U, lambda jt=jt: vproj(jt)))
        # xq ring has 4 slots (no recycling pressure): t0's late slices are
        # due before ih1 of pair0 (step 16), t1's before pair1 (step 48)
        for sl in range(4, 8):
            fillers.append((KNOBS['q47_t0'] + sl, U,
                            lambda sl=sl: qkproj('q', 0, sl)))
            fillers.append((KNOBS['q47_t1'] + sl, U,
                            lambda sl=sl: qkproj('q', 1, sl)))
        fillers = deque(sorted(fillers, key=lambda x: x[0]))
        late_fillers = deque()   # outproj st0-7, enabled after ih0 done

        # prologue: project both t's for the early xq slices (ring order) in
        # the DMA-wait gaps; kproj interleaves as soon as its DMA lands
        qkproj('q', 0, 0)
        qkproj('q', 1, 0)
        qkproj('q', 0, 1)
        qkproj('q', 1, 1)
        qkproj('k', 0, 0)
        qkproj('q', 0, 2)
        qkproj('q', 0, 3)

        # pair-major: all of pair0's attention first, so pair1's projections
        # land in the second half where the PE has slack
        steps = [(ih, pair, jt) for pair in range(2) for ih in range(2)
                 for jt in range(ST)]
        drainq = deque()        # (enq_step, ih, pair, hp, ic)
        remaining = {}          # (ih,pair) -> remaining drain units

        pending_fin = deque()   # software-pipeline finish_ic one ic behind

        def emit_drain(si, budget):
            _, dih, dpair, hp, ic, part = drainq.popleft()
            if part == 'p1':
                part1_unit(dih, dpair, hp, ic)
                return budget - 380.0
            drain_unit(dih, dpair, hp, ic)
            budget -= 200.0
            if hp == 1:
                pending_fin.append((dih, dpair, ic))
                if len(pending_fin) > 1:
                    finish_ic(*pending_fin.popleft())
                    budget -= 150.0
            remaining[(dih, dpair)] -= 1
            if remaining[(dih, dpair)] == 0:
                while pending_fin:
                    finish_ic(*pending_fin.popleft())
                boundary(dih, dpair)
                if (dih, dpair) == (0, 1):
                    for st in range(8):
                        late_fillers.append(lambda st=st: outproj(st))
            return budget

        p1units = [(ic, hp) for ic in range(8) for hp in range(2)]
        for si, (ih, pair, jt) in enumerate(steps):
            scores_exp(ih, pair, jt)
            budget = KNOBS['step_budget']
            if (ih, pair) == (1, 1) and jt >= JSPLIT:
                # last pair: part1 inside its own steps to shrink the tail
                for ic, hp in p1units[4 * (jt - JSPLIT):4 * (jt - JSPLIT) + 4]:
                    part1_unit(ih, pair, hp, ic)
                budget -= 4 * 380.0
            if jt == ST - 1:
                remaining[(ih, pair)] = 16
                if (ih, pair) != (1, 1):
                    for ic, hp in p1units:
                        drainq.append((si, ih, pair, hp, ic, 'p1'))
                for ic in range(8):
                    for hp in range(2):
                        drainq.append((si, ih, pair, hp, ic, 'p2'))
            # 1. hard-overdue fillers (ring safety) run regardless of budget
            while fillers and fillers[0][0] <= si - KNOBS['forced_age']:
                _, c, fn = fillers.popleft()
                fn()
                budget -= c
            # 2. due fillers, budget-gated
            while fillers and fillers[0][0] <= si and budget > 0:
                _, c, fn = fillers.popleft()
                fn()
                budget -= c
            # 3. drains: budget-gated, but a minimum rate once aged
            pops = 0
            while drainq and drainq[0][0] < si and pops < KNOBS['drain_cap'] \
                    and (budget > 0 or
                         si - drainq[0][0] >= KNOBS['drain_age']):
                budget = emit_drain(si, budget)
                pops += 1
            # 4. output projections of the finished half: guarantee one
            # per step so they don't pile into the tail
            if si >= KNOBS['late_start'] and late_fillers:
                late_fillers.popleft()()
                budget -= U
                while late_fillers and budget > 0:
                    late_fillers.popleft()()
                    budget -= U

        # tail: drain the last pair first (its boundary emits outproj
        # per-ic); leftover first-half outprojs only gate their own DMAs
        in_tail[0] = True
        si = len(steps)
        while drainq:
            emit_drain(si, 0.0)
        while late_fillers:
            late_fillers.popleft()()
        if DEBUG:
            nc.sync.dma_start(out=dbg['qT0'][:, :], in_=qT_sb[0])
            nc.sync.dma_start(out=dbg['kT0'][:, :], in_=kT_sb[0])
            nc.sync.dma_start(
                out=dbg['v'][:, :],
                in_=v_sb.rearrange("p a b c -> p (a b c)"))
            nc.sync.dma_start(out=dbg['attT0'][:, :], in_=attT[0])
            nc.sync.dma_start(out=dbg['attT1'][:, :], in_=attT[1])

    nc.finalize()
    return nc


def _pack_x(Xb):
    # [S, D] f32 -> [128, NSL*KC*SLW] bf16, slices sl-major, (c, s) inside
    a = Xb.reshape(NSL, SLW, KC, P).transpose(3, 0, 2, 1)
    return np.ascontiguousarray(a).reshape(P, NSL * KC * SLW).astype(BF16)


def kernel(Q, K, V, Wq, bq, Wk, bk, Wv, bv, Wo, bo):
    from concourse.bass_utils import run_bass_kernel_spmd

    f32 = np.float32
    Q = np.asarray(Q, f32)
    K = np.asarray(K, f32)
    V = np.asarray(V, f32)
    Wq = np.asarray(Wq, f32)
    Wk = np.asarray(Wk, f32)
    Wv = np.asarray(Wv, f32)
    Wo = np.asarray(Wo, f32)
    bq = np.asarray(bq, f32)
    bk = np.asarray(bk, f32)
    bv = np.asarray(bv, f32)
    bo = np.asarray(bo, f32)

    xp = {}
    for b in range(B):
        xp[('q', b)] = _pack_x(Q[b])
        xp[('k', b)] = _pack_x(K[b])
        xp[('v', b)] = _pack_x(V[b])

    ident = np.eye(P, dtype=BF16)

    def pack_w(Wslice):
        # [1024, 256] -> [128, KC, 256] (p, c, m) -> flat
        a = Wslice.reshape(KC, P, GD).transpose(1, 0, 2)
        return np.ascontiguousarray(a).reshape(P, KC * GD).astype(BF16)

    in_maps = []
    for c in range(NCORES):
        b, g = c // GROUPS, c % GROUPS
        sl = slice(g * GD, (g + 1) * GD)
        wo_a = Wo[sl, :].reshape(2, P, D).transpose(1, 0, 2)
        in_maps.append({
            "xq": xp[('q', b)],
            "xk": xp[('k', b)],
            "xv": xp[('v', b)],
            "wq": pack_w(Wq[:, sl]),
            "wk": pack_w(Wk[:, sl]),
            "wv": pack_w(Wv[:, sl]),
            "wo": np.ascontiguousarray(wo_a).reshape(P, 2 * D).astype(BF16),
            "bq": np.ascontiguousarray(bq[sl].reshape(2, P).T),
            "bk": np.ascontiguousarray(bk[sl].reshape(2, P).T),
            "ident": ident,
        })

    if "nc" not in _cached:
        _cached["nc"] = _build_bass()
    nc = _cached["nc"]

    try:
        res = run_bass_kernel_spmd(nc, in_maps, core_ids=list(range(NCORES)))
    except ModuleNotFoundError:
        os.environ["BASS_NEVER_TRACE"] = "1"
        res = run_bass_kernel_spmd(nc, in_maps, core_ids=list(range(NCORES)))
    if res.exec_time_ns is not None:
        print(f"HW exec time: {res.exec_time_ns} ns")

    bo_eff = (bv @ Wo + bo).astype(f32)
    out = np.zeros((B, S, D), f32)
    for c in range(NCORES):
        b = c // GROUPS
        out[b] += res.results[c]["out"].astype(f32)
    out += bo_eff
    return out
